# revision 33
# baseline (speedup 1.0000x reference)
"""AttentionPooling (segment softmax-pool) TRN2 kernel, 8-core SPMD. V2.

Self-contained: kernel(**inputs) -> np.ndarray [16384, 128] f32.

Math (shift-invariance of softmax; logits are O(1) so exp can't overflow):
  e_i   = exp(tanh(x_i @ W1 + b1) @ W2 + b2)
  out_g = (sum_{i in g} e_i x_i) / (sum_{i in g} e_i)

Sharding: graphs are split into 8 contiguous ranges with ~equal node counts
(each graph's nodes land on one core); each core computes its own rows of the
output; host concatenates.

V2 device algorithm per core: x streams once in each of two bf16 layouts
(transposed [128, M] for the MLP, natural [x|1]-tiles for pooling). Per 4096-row
chunk: 8 matmuls W1^T@xT -> PSUM, tanh on [128,1024] spans, 32 per-tile
(lhsT=hq, rhs=w2, N=1) matmuls -> per-node logits in [128,32], one Exp, then the
masked-e matrix me[p, t*S+s] = e*(seg==s) is built with two WIDE tensor_tensor
ops per chunk using stride-0 broadcast APs (vs per-tile tensor_scalar in V1).
Pool matmuls (lhsT=me-tile, rhs=[x|1]) accumulate [48,129] per window; a static
indicator matmul re-bins window-slots to segments; reciprocal+scale normalizes.
Logit and pool matmuls are interleaved to keep the PE streaming (HAM warm).
"""

import math

import numpy as np
import ml_dtypes

BF16 = ml_dtypes.bfloat16
FP8 = ml_dtypes.float8_e3m4   # xT stream dtype (W1 stays bf16: mixed matmul)

N_CORES = 8
N_GRAPHS = 16384
H = 128
TILE = 128
TPC = 32             # tiles per window
CHUNK = TILE * TPC   # 4096 rows
SLOTS = 64           # slot stride per window in stitch space
NW_STITCH = 8        # stitch window span (static)
NCHK = NW_STITCH * SLOTS // 128
MSLOT = 48           # active slot width (real data max is 36/window)
PAD_SEG = 128.0      # any value >= MSLOT, exactly representable in bf16

LAST_EXEC_NS = None
_PROGRAM_CACHE = {}


# ---------------------------------------------------------------- host prep
def _preprocess(x, batch, W1, b1, W2, b2, n_graphs):
    N = x.shape[0]
    counts = np.bincount(batch, minlength=n_graphs)
    cum = np.zeros(n_graphs + 1, dtype=np.int64)
    np.cumsum(counts, out=cum[1:])

    gsplit = [0]
    for c in range(1, N_CORES):
        t = round(c * N / N_CORES)
        g = int(np.searchsorted(cum, t))
        if g > 0 and abs(cum[g - 1] - t) <= abs(cum[g] - t):
            g -= 1
        g = max(g, gsplit[-1] + 1)
        gsplit.append(min(g, n_graphs - (N_CORES - c)))
    gsplit.append(n_graphs)
    gsplit = np.array(gsplit, dtype=np.int64)

    Mc = [int(cum[gsplit[c + 1]] - cum[gsplit[c]]) for c in range(N_CORES)]
    Gc = [int(gsplit[c + 1] - gsplit[c]) for c in range(N_CORES)]
    NWIN = max(NW_STITCH, math.ceil(max(Mc) / CHUNK))
    if NWIN % 2:
        NWIN += 1
    M_pad = NWIN * CHUNK
    NGRP = math.ceil(max(Gc) / 128)

    x = np.asarray(x, dtype=np.float32)
    batch = np.asarray(batch)

    cores = []
    minw = np.full((N_CORES, NGRP), 10 ** 9, dtype=np.int64)
    maxw = np.full((N_CORES, NGRP), -1, dtype=np.int64)
    for c in range(N_CORES):
        nlo = int(cum[gsplit[c]])
        nhi = int(cum[gsplit[c + 1]])
        m = Mc[c]
        bl = batch[nlo:nhi].astype(np.int64) - gsplit[c]
        wfs = np.zeros(NWIN, dtype=np.int64)
        for w in range(NWIN):
            wfs[w] = bl[w * CHUNK] if w * CHUNK < m else Gc[c]
        slots = bl - wfs[np.arange(m) // CHUNK]
        assert slots.min() >= 0 and slots.max() < MSLOT, (
            f"core {c}: window slot range {slots.min()}..{slots.max()}")

        seg = np.full(M_pad, PAD_SEG, dtype=np.float32)
        seg[:m] = slots.astype(np.float32)
        seg_img = np.ascontiguousarray(seg.reshape(-1, TILE).T.astype(BF16))

        nT = M_pad // TILE
        xn = np.zeros((M_pad, H + 1), dtype=BF16)
        xn[:m, :H] = x[nlo:nhi]
        xn[:, H] = 1.0
        xn_img = np.ascontiguousarray(
            xn.reshape(nT, TILE, H + 1).transpose(1, 0, 2)
            .reshape(TILE, nT * (H + 1)))

        xt = np.zeros((M_pad, H), dtype=FP8)
        xt[:m] = x[nlo:nhi].astype(FP8)
        xt_img = np.ascontiguousarray(xt.T)

        # one interleaved byte stream per chunk: [xT fp8 | xn bf16 | seg bf16]
        xall_img = np.ascontiguousarray(np.concatenate([
            xt_img.view(np.uint8).reshape(TILE, NWIN, CHUNK),
            xn_img.view(np.uint8).reshape(TILE, NWIN, 2 * TPC * (H + 1)),
            seg_img.view(np.uint8).reshape(TILE, NWIN, 2 * TPC),
        ], axis=2).reshape(TILE, -1))

        lo_g = cum[gsplit[c]:gsplit[c + 1]] - nlo
        hi_g = cum[gsplit[c] + 1:gsplit[c + 1] + 1] - nlo
        nonempty = hi_g > lo_g
        wlo_g = np.where(nonempty, lo_g // CHUNK, 0)
        whi_g = np.where(nonempty, np.maximum(hi_g - 1, 0) // CHUNK, 0)
        for Gi in range(NGRP):
            a, b = Gi * 128, min(Gi * 128 + 128, Gc[c])
            if a >= Gc[c]:
                continue
            ne = nonempty[a:b]
            if ne.any():
                minw[c, Gi] = wlo_g[a:b][ne].min()
                maxw[c, Gi] = whi_g[a:b][ne].max()
        cores.append(dict(m=m, gc=Gc[c], wfs=wfs, xall_img=xall_img,
                          nonempty=nonempty, wlo_g=wlo_g, whi_g=whi_g))

    wlo_shared = []
    for Gi in range(NGRP):
        mn = int(minw[:, Gi].min())
        if mn >= 10 ** 9:
            mn = 0
        mn -= mn % 2
        mn = max(0, min(mn, NWIN - NW_STITCH))
        wlo_shared.append(mn)
        mx = int(maxw[:, Gi].max())
        assert mx < 0 or mx - mn + 1 <= NW_STITCH, (
            f"group {Gi}: window span {mn}..{mx} exceeds {NW_STITCH}")

    for c in range(N_CORES):
        d = cores[c]
        ind = np.zeros((128, NGRP * NCHK * 128), dtype=BF16)
        for g in range(d["gc"]):
            if not d["nonempty"][g]:
                continue
            Gi = g // 128
            base_ws = wlo_shared[Gi] * SLOTS
            for w in range(int(d["wlo_g"][g]), int(d["whi_g"][g]) + 1):
                s = int(g - d["wfs"][w])
                wsl = w * SLOTS + s - base_ws
                assert 0 <= wsl < NW_STITCH * SLOTS
                ind[wsl % 128, (Gi * NCHK + wsl // 128) * 128 + (g - Gi * 128)] = 1.0
        d["ind_img"] = ind

    shared = dict(
        NWIN=NWIN, M_pad=M_pad, NGRP=NGRP, wlo_shared=wlo_shared,
        gsplit=gsplit, counts=counts,
        iota=np.ascontiguousarray(
            np.broadcast_to(np.arange(MSLOT, dtype=BF16), (128, MSLOT))),
        w1b=np.ascontiguousarray(np.asarray(W1).astype(BF16)),
        w2b=np.ascontiguousarray(np.asarray(W2).astype(BF16)),
        b1c=np.ascontiguousarray(np.asarray(b1).reshape(H, 1).astype(np.float32)),
        b2c=np.full((128, 1), np.asarray(b2).reshape(-1)[0], dtype=np.float32),
    )
    return shared, cores


# ---------------------------------------------------------------- program
def _build_program(NWIN, NGRP, wlo_shared,
                   xall_bufs=6, hq_bufs=3, me_bufs=4, me01_bufs=2):
    from contextlib import ExitStack
    import concourse.bacc as bacc
    import concourse.tile as tile
    from concourse import mybir

    M_pad = NWIN * CHUNK
    nT = M_pad // TILE
    NWCOL = NWIN * SLOTS // 128
    # interleaved per-chunk byte stream: [xT fp8 | xn bf16 | seg bf16]
    XAW = CHUNK + 2 * TPC * (H + 1) + 2 * TPC
    XT_OFF, XN_OFF, SG_OFF = 0, CHUNK, CHUNK + 2 * TPC * (H + 1)

    f32 = mybir.dt.float32
    bf16 = mybir.dt.bfloat16
    fp8 = mybir.dt.float8e3
    u8 = mybir.dt.uint8
    AF = mybir.ActivationFunctionType
    ALU = mybir.AluOpType

    nc = bacc.Bacc("TRN2", target_bir_lowering=False, debug=False,
                   enable_asserts=False, num_devices=N_CORES)
    xall_ap = nc.dram_tensor("xall", [128, NWIN * XAW], u8,
                             kind="ExternalInput").ap()
    iota_ap = nc.dram_tensor("iota", [128, MSLOT], bf16, kind="ExternalInput").ap()
    w1_ap = nc.dram_tensor("w1b", [128, H], bf16, kind="ExternalInput").ap()
    w2_ap = nc.dram_tensor("w2b", [128, 1], bf16, kind="ExternalInput").ap()
    b1_ap = nc.dram_tensor("b1c", [128, 1], f32, kind="ExternalInput").ap()
    b2_ap = nc.dram_tensor("b2c", [128, 1], f32, kind="ExternalInput").ap()
    ind_ap = nc.dram_tensor("ind", [128, NGRP * NCHK * 128], bf16,
                            kind="ExternalInput").ap()
    out_ap = nc.dram_tensor("out", [NGRP * 128, H], f32, kind="ExternalOutput").ap()

    with tile.TileContext(nc) as tc, ExitStack() as ctx:
        consts = ctx.enter_context(tc.tile_pool(name="consts", bufs=1))
        xall_pool = ctx.enter_context(tc.tile_pool(name="xall", bufs=xall_bufs))
        hq_pool = ctx.enter_context(tc.tile_pool(name="hqp", bufs=hq_bufs))
        ee_pool = ctx.enter_context(tc.tile_pool(name="eep", bufs=4))
        me01_pool = ctx.enter_context(tc.tile_pool(name="me01p", bufs=me01_bufs))
        me_pool = ctx.enter_context(tc.tile_pool(name="mep", bufs=me_bufs))
        wres_pool = ctx.enter_context(tc.tile_pool(name="wres", bufs=1))
        ind_pool = ctx.enter_context(tc.tile_pool(name="indp", bufs=4))
        r_pool = ctx.enter_context(tc.tile_pool(name="rp", bufs=2))
        ob_pool = ctx.enter_context(tc.tile_pool(name="obp", bufs=2))
        ht_psum = ctx.enter_context(tc.tile_pool(name="htps", bufs=5, space="PSUM"))
        pl_psum = ctx.enter_context(tc.tile_pool(name="plps", bufs=3, space="PSUM"))

        iota_t = consts.tile([128, MSLOT], bf16, tag="iota")
        nc.sync.dma_start(iota_t[:], iota_ap[:])
        w1_t = consts.tile([128, H], bf16, tag="w1")
        nc.sync.dma_start(w1_t[:], w1_ap[:])
        w2_t = consts.tile([128, 1], bf16, tag="w2")
        nc.sync.dma_start(w2_t[:], w2_ap[:])
        b1_t = consts.tile([128, 1], f32, tag="b1")
        nc.sync.dma_start(b1_t[:], b1_ap[:])
        b2_t = consts.tile([128, 1], f32, tag="b2")
        nc.sync.dma_start(b2_t[:], b2_ap[:])
        wres_cols = [wres_pool.tile([128, H + 1], bf16, name=f"wres{i}",
                                    tag=f"wres{i}")
                     for i in range(NWCOL)]
        for i in range(NWCOL):
            nc.vector.memset(wres_cols[i][:], 0.0)

        # emit group Gi's stitch right after its last window is flushed
        ready_groups = {}
        for Gi in range(NGRP):
            ready_groups.setdefault(wlo_shared[Gi] + NW_STITCH - 1, []).append(Gi)

        def emit_stitch(Gi):
            st = pl_psum.tile([128, H + 33], f32, tag="pl")
            for k in range(NCHK):
                it = ind_pool.tile([128, 128], bf16)
                nc.sync.dma_start(
                    it[:], ind_ap[:, (Gi * NCHK + k) * 128:(Gi * NCHK + k + 1) * 128])
                wc = wlo_shared[Gi] // 2 + k
                nc.tensor.matmul(st[:, 0:H + 1], lhsT=it[:], rhs=wres_cols[wc][:],
                                 start=(k == 0), stop=(k == NCHK - 1))
            r = r_pool.tile([128, 1], f32)
            nc.vector.reciprocal(r[:], st[:, H:H + 1])
            ob = ob_pool.tile([128, H], f32)
            nc.vector.tensor_scalar(ob[:], st[:, 0:H], r[:, 0:1], None,
                                    op0=ALU.mult)
            nc.sync.dma_start(out_ap[Gi * 128:(Gi + 1) * 128, :], ob[:])

        # ---- software pipeline over chunks -------------------------------
        # Iteration i emits, in order:
        #   PE : logit(i-1) x32 interleaved with MLP(i) x8   (dense stream)
        #   ACT: exp(i-1) first, then tanh(i) x4  (so the me chain isn't
        #        stuck behind 4us of tanh)
        #   DVE: me01(i-1), me(i-1)  (wide TTs via stride-0 broadcast APs)
        #   PE : pool(i-3) x32  (3-deep lag so me is always ready)
        st_dma = {}
        st_mlp = {}
        st_me = {}
        st_pl = {}
        LAG = 3
        for i in range(NWIN + LAG):
            do_mlp = i < NWIN
            do_lg = 1 <= i <= NWIN
            do_pl = LAG <= i <= NWIN + LAG - 1
            if do_mlp:
                xall = xall_pool.tile([128, XAW], u8, name=f"xa{i}", tag="xa")
                nc.sync.dma_start(xall[:], xall_ap[:, i * XAW:(i + 1) * XAW])
                st_dma[i] = xall
                hq = hq_pool.tile([128, CHUNK], bf16, name=f"hq{i}", tag="hq")
                st_mlp[i] = hq
            if do_lg:
                cl = i - 1
                hql = st_mlp[cl]
                # logits live in cols H+1 .. H+32 of chunk cl's pl tile
                plc = pl_psum.tile([128, H + 33], f32, name=f"pl{cl}", tag="pl")
                st_pl[cl] = plc
                lg = plc[:, H + 1:H + 33]
            if do_pl:
                cp = i - LAG
                xap, me = st_dma[cp], st_me[cp]
                strip = 64 * (cp % 2)
                pl = st_pl[cp]
            # PE: one dense sweep per iteration — logit(i-1) (N=1,
            # weight-load heavy), pool(i-3) (N=129) and MLP(i) (N=512)
            # interleaved so the array streams continuously (HAM stays warm).
            for t in range(TPC):
                if do_lg:
                    nc.tensor.matmul(lg[:, t:t + 1],
                                     lhsT=hql[:, t * 128:(t + 1) * 128],
                                     rhs=w2_t[:], start=True, stop=True,
                                     skip_group_check=True)
                if do_pl:
                    nc.tensor.matmul(
                        pl[strip:strip + MSLOT, 0:H + 1],
                        lhsT=me[:, t * MSLOT:(t + 1) * MSLOT],
                        rhs=xap[:, XN_OFF + 2 * t * (H + 1):
                                XN_OFF + 2 * (t + 1) * (H + 1)].bitcast(bf16),
                        start=(t == 0), stop=(t == TPC - 1),
                        tile_position=(0, strip),
                        skip_group_check=True)
                if do_mlp and t % 4 == 0:
                    q = t // 4
                    ht = ht_psum.tile([128, 512], f32,
                                      name=f"ht{i}_{q}", tag="ht")
                    nc.tensor.matmul(
                        ht[:], lhsT=w1_t[:],
                        rhs=xall[:, XT_OFF + q * 512:XT_OFF + (q + 1) * 512]
                        .bitcast(fp8),
                        start=True, stop=True)
                    nc.scalar.activation(
                        hq[:, q * 512:(q + 1) * 512], ht[:],
                        AF.Tanh, bias=b1_t[:, 0:1])
            if do_lg:
                # ACT: exp right after the logit sweep completes
                ee = ee_pool.tile([128, TPC], bf16, name=f"ee{cl}", tag="ee")
                nc.scalar.activation(ee[:], lg, AF.Exp, bias=b2_t[:, 0:1])
                # DVE: wide me build
                sgl = st_dma[cl]
                me01 = me01_pool.tile([128, TPC * MSLOT], bf16,
                                      name=f"me01_{cl}", tag="me01")
                sg_b = (sgl[:, SG_OFF:SG_OFF + 2 * TPC].bitcast(bf16)
                        .unsqueeze(2).broadcast_to((128, TPC, MSLOT)))
                iota_b = iota_t[:, :].unsqueeze(1).broadcast_to((128, TPC, MSLOT))
                me01_3d = me01[:].rearrange("p (t s) -> p t s", t=TPC)
                nc.vector.tensor_tensor(me01_3d, sg_b, iota_b, op=ALU.is_equal)
                men = me_pool.tile([128, TPC * MSLOT], bf16,
                                   name=f"me{cl}", tag="me")
                ee_b = ee[:, :].unsqueeze(2).broadcast_to((128, TPC, MSLOT))
                me_3d = men[:].rearrange("p (t s) -> p t s", t=TPC)
                nc.vector.tensor_tensor(me_3d, me01_3d, ee_b, op=ALU.mult)
                st_me[cl] = men
                st_mlp.pop(cl, None)
            if do_pl:
                nc.vector.tensor_copy(
                    wres_cols[cp // 2][strip:strip + MSLOT, :],
                    pl[strip:strip + MSLOT, 0:H + 1])
                for Gi in ready_groups.get(cp, ()):
                    emit_stitch(Gi)
                st_me.pop(cp, None)
                st_dma.pop(cp, None)
                st_pl.pop(cp, None)

    nc.compile()
    return nc


def kernel(x, batch, W1, b1, W2, b2):
    global LAST_EXEC_NS
    import os
    from concourse.bass_utils import run_bass_kernel_spmd

    x = np.asarray(x)
    batch = np.asarray(batch)
    shared, cores = _preprocess(x, batch, W1, b1, W2, b2, N_GRAPHS)

    key = (shared["NWIN"], shared["NGRP"], tuple(shared["wlo_shared"]))
    nc = _PROGRAM_CACHE.get(key)
    if nc is None:
        nc = _build_program(shared["NWIN"], shared["NGRP"], shared["wlo_shared"])
        _PROGRAM_CACHE[key] = nc

    in_maps = []
    for d in cores:
        in_maps.append({
            "xall": d["xall_img"],
            "iota": shared["iota"], "w1b": shared["w1b"], "w2b": shared["w2b"],
            "b1c": shared["b1c"], "b2c": shared["b2c"], "ind": d["ind_img"],
        })
    trace = os.environ.get("ATTNPOOL_TRACE", "0") == "1"
    tmpdir = os.environ.get("ATTNPOOL_TMPDIR") or None
    res = run_bass_kernel_spmd(nc, in_maps, core_ids=list(range(N_CORES)),
                               trace=trace, tmpdir=tmpdir)
    if res.exec_time_ns is not None:
        LAST_EXEC_NS = res.exec_time_ns

    out = np.zeros((N_GRAPHS, H), dtype=np.float32)
    gsplit = shared["gsplit"]
    for c, d in enumerate(cores):
        out[gsplit[c]:gsplit[c + 1]] = res.results[c]["out"][:d["gc"]]
    out[shared["counts"] == 0] = 0.0
    return out


# revision 35
# speedup vs baseline: 1.0802x; 1.0802x over previous
"""AttentionPooling (segment softmax-pool) TRN2 kernel, 8-core SPMD. V2.

Self-contained: kernel(**inputs) -> np.ndarray [16384, 128] f32.

Math (shift-invariance of softmax; logits are O(1) so exp can't overflow):
  e_i   = exp(tanh(x_i @ W1 + b1) @ W2 + b2)
  out_g = (sum_{i in g} e_i x_i) / (sum_{i in g} e_i)

Sharding: graphs are split into 8 contiguous ranges with ~equal node counts
(each graph's nodes land on one core); each core computes its own rows of the
output; host concatenates.

V2 device algorithm per core: x streams once in each of two bf16 layouts
(transposed [128, M] for the MLP, natural [x|1]-tiles for pooling). Per 4096-row
chunk: 8 matmuls W1^T@xT -> PSUM, tanh on [128,1024] spans, 32 per-tile
(lhsT=hq, rhs=w2, N=1) matmuls -> per-node logits in [128,32], one Exp, then the
masked-e matrix me[p, t*S+s] = e*(seg==s) is built with two WIDE tensor_tensor
ops per chunk using stride-0 broadcast APs (vs per-tile tensor_scalar in V1).
Pool matmuls (lhsT=me-tile, rhs=[x|1]) accumulate [48,129] per window; a static
indicator matmul re-bins window-slots to segments; reciprocal+scale normalizes.
Logit and pool matmuls are interleaved to keep the PE streaming (HAM warm).
"""

import math

import numpy as np
import ml_dtypes

BF16 = ml_dtypes.bfloat16
FP8 = ml_dtypes.float8_e3m4   # xT stream dtype (W1 stays bf16: mixed matmul)

N_CORES = 8
N_GRAPHS = 16384
H = 128
TILE = 128
TPC = 32             # tiles per window
CHUNK = TILE * TPC   # 4096 rows
SLOTS = 64           # slot stride per window in stitch space
NW_STITCH = 8        # stitch window span (static)
NCHK = NW_STITCH * SLOTS // 128
MSLOT = 48           # active slot width (real data max is 36/window)
PAD_SEG = 128.0      # any value >= MSLOT, exactly representable in bf16

LAST_EXEC_NS = None
_PROGRAM_CACHE = {}


# ---------------------------------------------------------------- host prep
def _preprocess(x, batch, W1, b1, W2, b2, n_graphs):
    N = x.shape[0]
    counts = np.bincount(batch, minlength=n_graphs)
    cum = np.zeros(n_graphs + 1, dtype=np.int64)
    np.cumsum(counts, out=cum[1:])

    gsplit = [0]
    for c in range(1, N_CORES):
        t = round(c * N / N_CORES)
        g = int(np.searchsorted(cum, t))
        if g > 0 and abs(cum[g - 1] - t) <= abs(cum[g] - t):
            g -= 1
        g = max(g, gsplit[-1] + 1)
        gsplit.append(min(g, n_graphs - (N_CORES - c)))
    gsplit.append(n_graphs)
    gsplit = np.array(gsplit, dtype=np.int64)

    Mc = [int(cum[gsplit[c + 1]] - cum[gsplit[c]]) for c in range(N_CORES)]
    Gc = [int(gsplit[c + 1] - gsplit[c]) for c in range(N_CORES)]
    NWIN = max(NW_STITCH, math.ceil(max(Mc) / CHUNK))
    if NWIN % 2:
        NWIN += 1
    M_pad = NWIN * CHUNK
    NGRP = math.ceil(max(Gc) / 128)

    x = np.asarray(x, dtype=np.float32)
    batch = np.asarray(batch)

    cores = []
    minw = np.full((N_CORES, NGRP), 10 ** 9, dtype=np.int64)
    maxw = np.full((N_CORES, NGRP), -1, dtype=np.int64)
    for c in range(N_CORES):
        nlo = int(cum[gsplit[c]])
        nhi = int(cum[gsplit[c + 1]])
        m = Mc[c]
        bl = batch[nlo:nhi].astype(np.int64) - gsplit[c]
        wfs = np.zeros(NWIN, dtype=np.int64)
        for w in range(NWIN):
            wfs[w] = bl[w * CHUNK] if w * CHUNK < m else Gc[c]
        slots = bl - wfs[np.arange(m) // CHUNK]
        assert slots.min() >= 0 and slots.max() < MSLOT, (
            f"core {c}: window slot range {slots.min()}..{slots.max()}")

        seg = np.full(M_pad, PAD_SEG, dtype=np.float32)
        seg[:m] = slots.astype(np.float32)
        seg_img = np.ascontiguousarray(seg.reshape(-1, TILE).T.astype(BF16))

        nT = M_pad // TILE
        xn = np.zeros((M_pad, H + 1), dtype=BF16)
        xn[:m, :H] = x[nlo:nhi]
        xn[:, H] = 1.0
        xn_img = np.ascontiguousarray(
            xn.reshape(nT, TILE, H + 1).transpose(1, 0, 2)
            .reshape(TILE, nT * (H + 1)))

        xt = np.zeros((M_pad, H), dtype=FP8)
        xt[:m] = x[nlo:nhi].astype(FP8)
        xt_img = np.ascontiguousarray(xt.T)

        # one interleaved byte stream per chunk: [xT fp8 | xn bf16 | seg bf16]
        xall_img = np.ascontiguousarray(np.concatenate([
            xt_img.view(np.uint8).reshape(TILE, NWIN, CHUNK),
            xn_img.view(np.uint8).reshape(TILE, NWIN, 2 * TPC * (H + 1)),
            seg_img.view(np.uint8).reshape(TILE, NWIN, 2 * TPC),
        ], axis=2).reshape(TILE, -1))

        lo_g = cum[gsplit[c]:gsplit[c + 1]] - nlo
        hi_g = cum[gsplit[c] + 1:gsplit[c + 1] + 1] - nlo
        nonempty = hi_g > lo_g
        wlo_g = np.where(nonempty, lo_g // CHUNK, 0)
        whi_g = np.where(nonempty, np.maximum(hi_g - 1, 0) // CHUNK, 0)
        for Gi in range(NGRP):
            a, b = Gi * 128, min(Gi * 128 + 128, Gc[c])
            if a >= Gc[c]:
                continue
            ne = nonempty[a:b]
            if ne.any():
                minw[c, Gi] = wlo_g[a:b][ne].min()
                maxw[c, Gi] = whi_g[a:b][ne].max()
        cores.append(dict(m=m, gc=Gc[c], wfs=wfs, xall_img=xall_img,
                          nonempty=nonempty, wlo_g=wlo_g, whi_g=whi_g))

    wlo_shared = []
    for Gi in range(NGRP):
        mn = int(minw[:, Gi].min())
        if mn >= 10 ** 9:
            mn = 0
        mn -= mn % 2
        mn = max(0, min(mn, NWIN - NW_STITCH))
        wlo_shared.append(mn)
        mx = int(maxw[:, Gi].max())
        assert mx < 0 or mx - mn + 1 <= NW_STITCH, (
            f"group {Gi}: window span {mn}..{mx} exceeds {NW_STITCH}")

    for c in range(N_CORES):
        d = cores[c]
        ind = np.zeros((128, NGRP * NCHK * 128), dtype=BF16)
        for g in range(d["gc"]):
            if not d["nonempty"][g]:
                continue
            Gi = g // 128
            base_ws = wlo_shared[Gi] * SLOTS
            for w in range(int(d["wlo_g"][g]), int(d["whi_g"][g]) + 1):
                s = int(g - d["wfs"][w])
                wsl = w * SLOTS + s - base_ws
                assert 0 <= wsl < NW_STITCH * SLOTS
                ind[wsl % 128, (Gi * NCHK + wsl // 128) * 128 + (g - Gi * 128)] = 1.0
        d["ind_img"] = ind

    shared = dict(
        NWIN=NWIN, M_pad=M_pad, NGRP=NGRP, wlo_shared=wlo_shared,
        gsplit=gsplit, counts=counts,
        iota=np.ascontiguousarray(
            np.broadcast_to(np.arange(MSLOT, dtype=BF16), (128, MSLOT))),
        w1b=np.ascontiguousarray(np.asarray(W1).astype(BF16)),
        w2b=np.ascontiguousarray(np.asarray(W2).astype(BF16)),
        b1c=np.ascontiguousarray(np.asarray(b1).reshape(H, 1).astype(np.float32)),
        b2c=np.full((128, 1), np.asarray(b2).reshape(-1)[0], dtype=np.float32),
    )
    return shared, cores


# ---------------------------------------------------------------- program
def _build_program(NWIN, NGRP, wlo_shared,
                   xall_bufs=6, hq_bufs=3, me_bufs=4, me01_bufs=2):
    from contextlib import ExitStack
    import concourse.bacc as bacc
    import concourse.tile as tile
    from concourse import mybir

    M_pad = NWIN * CHUNK
    nT = M_pad // TILE
    NWCOL = NWIN * SLOTS // 128
    # interleaved per-chunk byte stream: [xT fp8 | xn bf16 | seg bf16]
    XAW = CHUNK + 2 * TPC * (H + 1) + 2 * TPC
    XT_OFF, XN_OFF, SG_OFF = 0, CHUNK, CHUNK + 2 * TPC * (H + 1)

    f32 = mybir.dt.float32
    bf16 = mybir.dt.bfloat16
    fp8 = mybir.dt.float8e3
    u8 = mybir.dt.uint8
    AF = mybir.ActivationFunctionType
    ALU = mybir.AluOpType

    nc = bacc.Bacc("TRN2", target_bir_lowering=False, debug=False,
                   enable_asserts=False, num_devices=N_CORES)
    xall_ap = nc.dram_tensor("xall", [128, NWIN * XAW], u8,
                             kind="ExternalInput").ap()
    iota_ap = nc.dram_tensor("iota", [128, MSLOT], bf16, kind="ExternalInput").ap()
    w1_ap = nc.dram_tensor("w1b", [128, H], bf16, kind="ExternalInput").ap()
    w2_ap = nc.dram_tensor("w2b", [128, 1], bf16, kind="ExternalInput").ap()
    b1_ap = nc.dram_tensor("b1c", [128, 1], f32, kind="ExternalInput").ap()
    b2_ap = nc.dram_tensor("b2c", [128, 1], f32, kind="ExternalInput").ap()
    ind_ap = nc.dram_tensor("ind", [128, NGRP * NCHK * 128], bf16,
                            kind="ExternalInput").ap()
    out_ap = nc.dram_tensor("out", [NGRP * 128, H], f32, kind="ExternalOutput").ap()

    with tile.TileContext(nc) as tc, ExitStack() as ctx:
        consts = ctx.enter_context(tc.tile_pool(name="consts", bufs=1))
        xall_pool = ctx.enter_context(tc.tile_pool(name="xall", bufs=xall_bufs))
        hq_pool = ctx.enter_context(tc.tile_pool(name="hqp", bufs=hq_bufs))
        ee_pool = ctx.enter_context(tc.tile_pool(name="eep", bufs=4))
        me01_pool = ctx.enter_context(tc.tile_pool(name="me01p", bufs=me01_bufs))
        me_pool = ctx.enter_context(tc.tile_pool(name="mep", bufs=me_bufs))
        wres_pool = ctx.enter_context(tc.tile_pool(name="wres", bufs=1))
        ind_pool = ctx.enter_context(tc.tile_pool(name="indp", bufs=4))
        r_pool = ctx.enter_context(tc.tile_pool(name="rp", bufs=2))
        ob_pool = ctx.enter_context(tc.tile_pool(name="obp", bufs=2))
        ht_psum = ctx.enter_context(tc.tile_pool(name="htps", bufs=5, space="PSUM"))
        pl_psum = ctx.enter_context(tc.tile_pool(name="plps", bufs=3, space="PSUM"))

        iota_t = consts.tile([128, MSLOT], bf16, tag="iota")
        nc.sync.dma_start(iota_t[:], iota_ap[:])
        w1_t = consts.tile([128, H], bf16, tag="w1")
        nc.sync.dma_start(w1_t[:], w1_ap[:])
        w2_t = consts.tile([128, 1], bf16, tag="w2")
        nc.sync.dma_start(w2_t[:], w2_ap[:])
        b1_t = consts.tile([128, 1], f32, tag="b1")
        nc.sync.dma_start(b1_t[:], b1_ap[:])
        b2_t = consts.tile([128, 1], f32, tag="b2")
        nc.sync.dma_start(b2_t[:], b2_ap[:])
        wres_cols = [wres_pool.tile([128, H + 1], bf16, name=f"wres{i}",
                                    tag=f"wres{i}")
                     for i in range(NWCOL)]
        for i in range(NWCOL):
            nc.vector.memset(wres_cols[i][:], 0.0)

        # emit group Gi's stitch right after its last window is flushed
        ready_groups = {}
        for Gi in range(NGRP):
            ready_groups.setdefault(wlo_shared[Gi] + NW_STITCH - 1, []).append(Gi)

        def emit_stitch(Gi):
            st = pl_psum.tile([128, H + 33], f32, tag="pl")
            for k in range(NCHK):
                it = ind_pool.tile([128, 128], bf16)
                nc.sync.dma_start(
                    it[:], ind_ap[:, (Gi * NCHK + k) * 128:(Gi * NCHK + k + 1) * 128])
                wc = wlo_shared[Gi] // 2 + k
                nc.tensor.matmul(st[:, 0:H + 1], lhsT=it[:], rhs=wres_cols[wc][:],
                                 start=(k == 0), stop=(k == NCHK - 1))
            r = r_pool.tile([128, 1], f32)
            nc.vector.reciprocal(r[:], st[:, H:H + 1])
            ob = ob_pool.tile([128, H], f32)
            nc.vector.tensor_scalar(ob[:], st[:, 0:H], r[:, 0:1], None,
                                    op0=ALU.mult)
            nc.sync.dma_start(out_ap[Gi * 128:(Gi + 1) * 128, :], ob[:])

        # ---- software pipeline over chunks -------------------------------
        # Iteration i emits, in order:
        #   PE : logit(i-1) x32 interleaved with MLP(i) x8   (dense stream)
        #   ACT: exp(i-1) first, then tanh(i) x4  (so the me chain isn't
        #        stuck behind 4us of tanh)
        #   DVE: me01(i-1), me(i-1)  (wide TTs via stride-0 broadcast APs)
        #   PE : pool(i-3) x32  (3-deep lag so me is always ready)
        st_dma = {}
        st_mlp = {}
        st_me = {}
        st_pl = {}
        LAG = 3
        for i in range(NWIN + LAG):
            do_mlp = i < NWIN
            do_lg = 1 <= i <= NWIN
            do_pl = LAG <= i <= NWIN + LAG - 1
            if do_mlp:
                xall = xall_pool.tile([128, XAW], u8, name=f"xa{i}", tag="xa")
                nc.sync.dma_start(xall[:], xall_ap[:, i * XAW:(i + 1) * XAW])
                st_dma[i] = xall
                hq = hq_pool.tile([128, CHUNK], bf16, name=f"hq{i}", tag="hq")
                st_mlp[i] = hq
            if do_lg:
                cl = i - 1
                hql = st_mlp[cl]
                # logits live in cols H+1 .. H+32 of chunk cl's pl tile
                plc = pl_psum.tile([128, H + 33], f32, name=f"pl{cl}", tag="pl")
                st_pl[cl] = plc
                lg = plc[:, H + 1:H + 33]
            # PE: interleave logit(i-1) (weight-load heavy, N=1) with MLP(i)
            # (N=512 streams) so the array streams while hq weights load.
            for t in range(TPC):
                if do_lg:
                    nc.tensor.matmul(lg[:, t:t + 1],
                                     lhsT=hql[:, t * 128:(t + 1) * 128],
                                     rhs=w2_t[:], start=True, stop=True,
                                     skip_group_check=True)
                if do_mlp and t % 4 == 0:
                    q = t // 4
                    ht = ht_psum.tile([128, 512], f32,
                                      name=f"ht{i}_{q}", tag="ht")
                    nc.tensor.matmul(
                        ht[:], lhsT=w1_t[:],
                        rhs=xall[:, XT_OFF + q * 512:XT_OFF + (q + 1) * 512]
                        .bitcast(fp8),
                        start=True, stop=True)
                    nc.scalar.activation(
                        hq[:, q * 512:(q + 1) * 512], ht[:],
                        AF.Tanh, bias=b1_t[:, 0:1])
            if do_lg:
                # ACT: exp right after the logit sweep completes
                ee = ee_pool.tile([128, TPC], bf16, name=f"ee{cl}", tag="ee")
                nc.scalar.activation(ee[:], lg, AF.Exp, bias=b2_t[:, 0:1])
                # DVE: wide me build
                sgl = st_dma[cl]
                me01 = me01_pool.tile([128, TPC * MSLOT], bf16,
                                      name=f"me01_{cl}", tag="me01")
                sg_b = (sgl[:, SG_OFF:SG_OFF + 2 * TPC].bitcast(bf16)
                        .unsqueeze(2).broadcast_to((128, TPC, MSLOT)))
                iota_b = iota_t[:, :].unsqueeze(1).broadcast_to((128, TPC, MSLOT))
                me01_3d = me01[:].rearrange("p (t s) -> p t s", t=TPC)
                nc.vector.tensor_tensor(me01_3d, sg_b, iota_b, op=ALU.is_equal)
                men = me_pool.tile([128, TPC * MSLOT], bf16,
                                   name=f"me{cl}", tag="me")
                ee_b = ee[:, :].unsqueeze(2).broadcast_to((128, TPC, MSLOT))
                me_3d = men[:].rearrange("p (t s) -> p t s", t=TPC)
                nc.vector.tensor_tensor(me_3d, me01_3d, ee_b, op=ALU.mult)
                st_me[cl] = men
                st_mlp.pop(cl, None)
            if do_pl:
                cp = i - LAG
                xap, me = st_dma[cp], st_me[cp]
                strip = 64 * (cp % 2)
                pl = st_pl[cp]
                for t in range(TPC):
                    nc.tensor.matmul(
                        pl[strip:strip + MSLOT, 0:H + 1],
                        lhsT=me[:, t * MSLOT:(t + 1) * MSLOT],
                        rhs=xap[:, XN_OFF + 2 * t * (H + 1):
                                XN_OFF + 2 * (t + 1) * (H + 1)].bitcast(bf16),
                        start=(t == 0), stop=(t == TPC - 1),
                        tile_position=(0, strip),
                        skip_group_check=True)
                nc.vector.tensor_copy(
                    wres_cols[cp // 2][strip:strip + MSLOT, :],
                    pl[strip:strip + MSLOT, 0:H + 1])
                for Gi in ready_groups.get(cp, ()):
                    emit_stitch(Gi)
                st_me.pop(cp, None)
                st_dma.pop(cp, None)
                st_pl.pop(cp, None)

    nc.compile()
    return nc


def kernel(x, batch, W1, b1, W2, b2):
    global LAST_EXEC_NS
    import os
    from concourse.bass_utils import run_bass_kernel_spmd

    x = np.asarray(x)
    batch = np.asarray(batch)
    shared, cores = _preprocess(x, batch, W1, b1, W2, b2, N_GRAPHS)

    key = (shared["NWIN"], shared["NGRP"], tuple(shared["wlo_shared"]))
    nc = _PROGRAM_CACHE.get(key)
    if nc is None:
        nc = _build_program(shared["NWIN"], shared["NGRP"], shared["wlo_shared"])
        _PROGRAM_CACHE[key] = nc

    in_maps = []
    for d in cores:
        in_maps.append({
            "xall": d["xall_img"],
            "iota": shared["iota"], "w1b": shared["w1b"], "w2b": shared["w2b"],
            "b1c": shared["b1c"], "b2c": shared["b2c"], "ind": d["ind_img"],
        })
    trace = os.environ.get("ATTNPOOL_TRACE", "0") == "1"
    tmpdir = os.environ.get("ATTNPOOL_TMPDIR") or None
    res = run_bass_kernel_spmd(nc, in_maps, core_ids=list(range(N_CORES)),
                               trace=trace, tmpdir=tmpdir)
    if res.exec_time_ns is not None:
        LAST_EXEC_NS = res.exec_time_ns

    out = np.zeros((N_GRAPHS, H), dtype=np.float32)
    gsplit = shared["gsplit"]
    for c, d in enumerate(cores):
        out[gsplit[c]:gsplit[c + 1]] = res.results[c]["out"][:d["gc"]]
    out[shared["counts"] == 0] = 0.0
    return out


# revision 36
# speedup vs baseline: 1.2726x; 1.1781x over previous
"""AttentionPooling (segment softmax-pool) TRN2 kernel, 8-core SPMD. V2.

Self-contained: kernel(**inputs) -> np.ndarray [16384, 128] f32.

Math (shift-invariance of softmax; logits are O(1) so exp can't overflow):
  e_i   = exp(tanh(x_i @ W1 + b1) @ W2 + b2)
  out_g = (sum_{i in g} e_i x_i) / (sum_{i in g} e_i)

Sharding: graphs are split into 8 contiguous ranges with ~equal node counts
(each graph's nodes land on one core); each core computes its own rows of the
output; host concatenates.

V2 device algorithm per core: x streams once in each of two bf16 layouts
(transposed [128, M] for the MLP, natural [x|1]-tiles for pooling). Per 4096-row
chunk: 8 matmuls W1^T@xT -> PSUM, tanh on [128,1024] spans, 32 per-tile
(lhsT=hq, rhs=w2, N=1) matmuls -> per-node logits in [128,32], one Exp, then the
masked-e matrix me[p, t*S+s] = e*(seg==s) is built with two WIDE tensor_tensor
ops per chunk using stride-0 broadcast APs (vs per-tile tensor_scalar in V1).
Pool matmuls (lhsT=me-tile, rhs=[x|1]) accumulate [48,129] per window; a static
indicator matmul re-bins window-slots to segments; reciprocal+scale normalizes.
Logit and pool matmuls are interleaved to keep the PE streaming (HAM warm).
"""

import math

import numpy as np
import ml_dtypes

BF16 = ml_dtypes.bfloat16
FP8 = ml_dtypes.float8_e3m4   # xT stream dtype (W1 stays bf16: mixed matmul)

N_CORES = 8
N_GRAPHS = 16384
H = 128
TILE = 128
TPC = 32             # tiles per window
CHUNK = TILE * TPC   # 4096 rows
SLOTS = 64           # slot stride per window in stitch space
NW_STITCH = 8        # stitch window span (static)
NCHK = NW_STITCH * SLOTS // 128
MSLOT = 48           # active slot width (real data max is 36/window)
PAD_SEG = 128.0      # any value >= MSLOT, exactly representable in bf16

LAST_EXEC_NS = None
_PROGRAM_CACHE = {}


# ---------------------------------------------------------------- host prep
def _preprocess(x, batch, W1, b1, W2, b2, n_graphs):
    N = x.shape[0]
    counts = np.bincount(batch, minlength=n_graphs)
    cum = np.zeros(n_graphs + 1, dtype=np.int64)
    np.cumsum(counts, out=cum[1:])

    gsplit = [0]
    for c in range(1, N_CORES):
        t = round(c * N / N_CORES)
        g = int(np.searchsorted(cum, t))
        if g > 0 and abs(cum[g - 1] - t) <= abs(cum[g] - t):
            g -= 1
        g = max(g, gsplit[-1] + 1)
        gsplit.append(min(g, n_graphs - (N_CORES - c)))
    gsplit.append(n_graphs)
    gsplit = np.array(gsplit, dtype=np.int64)

    Mc = [int(cum[gsplit[c + 1]] - cum[gsplit[c]]) for c in range(N_CORES)]
    Gc = [int(gsplit[c + 1] - gsplit[c]) for c in range(N_CORES)]
    NWIN = max(NW_STITCH, math.ceil(max(Mc) / CHUNK))
    if NWIN % 2:
        NWIN += 1
    M_pad = NWIN * CHUNK
    NGRP = math.ceil(max(Gc) / 128)

    x = np.asarray(x, dtype=np.float32)
    batch = np.asarray(batch)

    cores = []
    minw = np.full((N_CORES, NGRP), 10 ** 9, dtype=np.int64)
    maxw = np.full((N_CORES, NGRP), -1, dtype=np.int64)
    for c in range(N_CORES):
        nlo = int(cum[gsplit[c]])
        nhi = int(cum[gsplit[c + 1]])
        m = Mc[c]
        bl = batch[nlo:nhi].astype(np.int64) - gsplit[c]
        wfs = np.zeros(NWIN, dtype=np.int64)
        for w in range(NWIN):
            wfs[w] = bl[w * CHUNK] if w * CHUNK < m else Gc[c]
        slots = bl - wfs[np.arange(m) // CHUNK]
        assert slots.min() >= 0 and slots.max() < MSLOT, (
            f"core {c}: window slot range {slots.min()}..{slots.max()}")

        seg = np.full(M_pad, PAD_SEG, dtype=np.float32)
        seg[:m] = slots.astype(np.float32)
        seg_img = np.ascontiguousarray(seg.reshape(-1, TILE).T.astype(BF16))

        nT = M_pad // TILE
        xn = np.zeros((M_pad, H + 1), dtype=BF16)
        xn[:m, :H] = x[nlo:nhi]
        xn[:, H] = 1.0
        xn_img = np.ascontiguousarray(
            xn.reshape(nT, TILE, H + 1).transpose(1, 0, 2)
            .reshape(TILE, nT * (H + 1)))

        xt = np.zeros((M_pad, H), dtype=FP8)
        xt[:m] = x[nlo:nhi].astype(FP8)
        xt_img = np.ascontiguousarray(xt.T)

        # one interleaved byte stream per chunk: [xT fp8 | xn bf16 | seg bf16]
        xall_img = np.ascontiguousarray(np.concatenate([
            xt_img.view(np.uint8).reshape(TILE, NWIN, CHUNK),
            xn_img.view(np.uint8).reshape(TILE, NWIN, 2 * TPC * (H + 1)),
            seg_img.view(np.uint8).reshape(TILE, NWIN, 2 * TPC),
        ], axis=2).reshape(TILE, -1))

        lo_g = cum[gsplit[c]:gsplit[c + 1]] - nlo
        hi_g = cum[gsplit[c] + 1:gsplit[c + 1] + 1] - nlo
        nonempty = hi_g > lo_g
        wlo_g = np.where(nonempty, lo_g // CHUNK, 0)
        whi_g = np.where(nonempty, np.maximum(hi_g - 1, 0) // CHUNK, 0)
        for Gi in range(NGRP):
            a, b = Gi * 128, min(Gi * 128 + 128, Gc[c])
            if a >= Gc[c]:
                continue
            ne = nonempty[a:b]
            if ne.any():
                minw[c, Gi] = wlo_g[a:b][ne].min()
                maxw[c, Gi] = whi_g[a:b][ne].max()
        cores.append(dict(m=m, gc=Gc[c], wfs=wfs, xall_img=xall_img,
                          nonempty=nonempty, wlo_g=wlo_g, whi_g=whi_g))

    wlo_shared = []
    for Gi in range(NGRP):
        mn = int(minw[:, Gi].min())
        if mn >= 10 ** 9:
            mn = 0
        mn -= mn % 2
        mn = max(0, min(mn, NWIN - NW_STITCH))
        wlo_shared.append(mn)
        mx = int(maxw[:, Gi].max())
        assert mx < 0 or mx - mn + 1 <= NW_STITCH, (
            f"group {Gi}: window span {mn}..{mx} exceeds {NW_STITCH}")

    for c in range(N_CORES):
        d = cores[c]
        ind = np.zeros((128, NGRP * NCHK * 128), dtype=np.float32)
        for g in range(d["gc"]):
            if not d["nonempty"][g]:
                continue
            Gi = g // 128
            base_ws = wlo_shared[Gi] * SLOTS
            for w in range(int(d["wlo_g"][g]), int(d["whi_g"][g]) + 1):
                s = int(g - d["wfs"][w])
                wsl = w * SLOTS + s - base_ws
                assert 0 <= wsl < NW_STITCH * SLOTS
                ind[wsl % 128, (Gi * NCHK + wsl // 128) * 128 + (g - Gi * 128)] = 1.0
        d["ind_img"] = ind

    shared = dict(
        NWIN=NWIN, M_pad=M_pad, NGRP=NGRP, wlo_shared=wlo_shared,
        gsplit=gsplit, counts=counts,
        iota=np.ascontiguousarray(
            np.broadcast_to(np.arange(MSLOT, dtype=BF16), (128, MSLOT))),
        w1b=np.ascontiguousarray(np.asarray(W1).astype(BF16)),
        w2b=np.ascontiguousarray(np.asarray(W2).astype(BF16)),
        b1c=np.ascontiguousarray(np.asarray(b1).reshape(H, 1).astype(np.float32)),
        b2c=np.full((128, 1), np.asarray(b2).reshape(-1)[0], dtype=np.float32),
    )
    return shared, cores


# ---------------------------------------------------------------- program
def _build_program(NWIN, NGRP, wlo_shared,
                   xall_bufs=6, hq_bufs=3, me_bufs=4, me01_bufs=2):
    from contextlib import ExitStack
    import concourse.bacc as bacc
    import concourse.tile as tile
    from concourse import mybir

    M_pad = NWIN * CHUNK
    nT = M_pad // TILE
    NWCOL = NWIN * SLOTS // 128
    # interleaved per-chunk byte stream: [xT fp8 | xn bf16 | seg bf16]
    XAW = CHUNK + 2 * TPC * (H + 1) + 2 * TPC
    XT_OFF, XN_OFF, SG_OFF = 0, CHUNK, CHUNK + 2 * TPC * (H + 1)

    f32 = mybir.dt.float32
    bf16 = mybir.dt.bfloat16
    fp8 = mybir.dt.float8e3
    u8 = mybir.dt.uint8
    AF = mybir.ActivationFunctionType
    ALU = mybir.AluOpType

    nc = bacc.Bacc("TRN2", target_bir_lowering=False, debug=False,
                   enable_asserts=False, num_devices=N_CORES)
    xall_ap = nc.dram_tensor("xall", [128, NWIN * XAW], u8,
                             kind="ExternalInput").ap()
    iota_ap = nc.dram_tensor("iota", [128, MSLOT], bf16, kind="ExternalInput").ap()
    w1_ap = nc.dram_tensor("w1b", [128, H], bf16, kind="ExternalInput").ap()
    w2_ap = nc.dram_tensor("w2b", [128, 1], bf16, kind="ExternalInput").ap()
    b1_ap = nc.dram_tensor("b1c", [128, 1], f32, kind="ExternalInput").ap()
    b2_ap = nc.dram_tensor("b2c", [128, 1], f32, kind="ExternalInput").ap()
    ind_ap = nc.dram_tensor("ind", [128, NGRP * NCHK * 128], f32,
                            kind="ExternalInput").ap()
    out_ap = nc.dram_tensor("out", [NGRP * 128, H], f32, kind="ExternalOutput").ap()

    with tile.TileContext(nc) as tc, ExitStack() as ctx:
        consts = ctx.enter_context(tc.tile_pool(name="consts", bufs=1))
        xall_pool = ctx.enter_context(tc.tile_pool(name="xall", bufs=xall_bufs))
        hq_pool = ctx.enter_context(tc.tile_pool(name="hqp", bufs=hq_bufs))
        ee_pool = ctx.enter_context(tc.tile_pool(name="eep", bufs=4))
        me01_pool = ctx.enter_context(tc.tile_pool(name="me01p", bufs=me01_bufs))
        me_pool = ctx.enter_context(tc.tile_pool(name="mep", bufs=me_bufs))
        wres_pool = ctx.enter_context(tc.tile_pool(name="wres", bufs=1))
        ind_pool = ctx.enter_context(tc.tile_pool(name="indp", bufs=4))
        r_pool = ctx.enter_context(tc.tile_pool(name="rp", bufs=2))
        ob_pool = ctx.enter_context(tc.tile_pool(name="obp", bufs=2))
        ht_psum = ctx.enter_context(tc.tile_pool(name="htps", bufs=5, space="PSUM"))
        pl_psum = ctx.enter_context(tc.tile_pool(name="plps", bufs=3, space="PSUM"))

        iota_t = consts.tile([128, MSLOT], bf16, tag="iota")
        nc.sync.dma_start(iota_t[:], iota_ap[:])
        w1_t = consts.tile([128, H], bf16, tag="w1")
        nc.sync.dma_start(w1_t[:], w1_ap[:])
        w2_t = consts.tile([128, 1], bf16, tag="w2")
        nc.sync.dma_start(w2_t[:], w2_ap[:])
        b1_t = consts.tile([128, 1], f32, tag="b1")
        nc.sync.dma_start(b1_t[:], b1_ap[:])
        b2_t = consts.tile([128, 1], f32, tag="b2")
        nc.sync.dma_start(b2_t[:], b2_ap[:])
        wres_cols = [wres_pool.tile([128, H + 1], f32, name=f"wres{i}",
                                    tag=f"wres{i}")
                     for i in range(NWCOL)]
        for i in range(NWCOL):
            nc.vector.memset(wres_cols[i][:], 0.0)

        # emit group Gi's stitch right after its last window is flushed
        ready_groups = {}
        for Gi in range(NGRP):
            ready_groups.setdefault(wlo_shared[Gi] + NW_STITCH - 1, []).append(Gi)

        def emit_stitch(Gi):
            st = pl_psum.tile([128, H + 33], f32, tag="pl")
            for k in range(NCHK):
                it = ind_pool.tile([128, 128], f32)
                nc.sync.dma_start(
                    it[:], ind_ap[:, (Gi * NCHK + k) * 128:(Gi * NCHK + k + 1) * 128])
                wc = wlo_shared[Gi] // 2 + k
                nc.tensor.matmul(st[:, 0:H + 1], lhsT=it[:], rhs=wres_cols[wc][:],
                                 start=(k == 0), stop=(k == NCHK - 1))
            r = r_pool.tile([128, 1], f32)
            nc.vector.reciprocal(r[:], st[:, H:H + 1])
            ob = ob_pool.tile([128, H], f32)
            nc.vector.tensor_scalar(ob[:], st[:, 0:H], r[:, 0:1], None,
                                    op0=ALU.mult)
            nc.sync.dma_start(out_ap[Gi * 128:(Gi + 1) * 128, :], ob[:])

        # ---- software pipeline over chunks -------------------------------
        # Iteration i emits, in order:
        #   PE : logit(i-1) x32 interleaved with MLP(i) x8   (dense stream)
        #   ACT: exp(i-1) first, then tanh(i) x4  (so the me chain isn't
        #        stuck behind 4us of tanh)
        #   DVE: me01(i-1), me(i-1)  (wide TTs via stride-0 broadcast APs)
        #   PE : pool(i-3) x32  (3-deep lag so me is always ready)
        st_dma = {}
        st_mlp = {}
        st_me = {}
        st_pl = {}
        LAG = 3
        for i in range(NWIN + LAG):
            do_mlp = i < NWIN
            do_lg = 1 <= i <= NWIN
            do_pl = LAG <= i <= NWIN + LAG - 1
            if do_mlp:
                xall = xall_pool.tile([128, XAW], u8, name=f"xa{i}", tag="xa")
                nc.sync.dma_start(xall[:], xall_ap[:, i * XAW:(i + 1) * XAW])
                st_dma[i] = xall
                hq = hq_pool.tile([128, CHUNK], bf16, name=f"hq{i}", tag="hq")
                st_mlp[i] = hq
            if do_lg:
                cl = i - 1
                hql = st_mlp[cl]
                # logits live in cols H+1 .. H+32 of chunk cl's pl tile
                plc = pl_psum.tile([128, H + 33], f32, name=f"pl{cl}", tag="pl")
                st_pl[cl] = plc
                lg = plc[:, H + 1:H + 33]
            # PE: interleave logit(i-1) (weight-load heavy, N=1) with MLP(i)
            # (N=512 streams) so the array streams while hq weights load.
            for t in range(TPC):
                if do_lg:
                    nc.tensor.matmul(lg[:, t:t + 1],
                                     lhsT=hql[:, t * 128:(t + 1) * 128],
                                     rhs=w2_t[:], start=True, stop=True,
                                     skip_group_check=True)
                if do_mlp and t % 4 == 0:
                    q = t // 4
                    ht = ht_psum.tile([128, 512], f32,
                                      name=f"ht{i}_{q}", tag="ht")
                    nc.tensor.matmul(
                        ht[:], lhsT=w1_t[:],
                        rhs=xall[:, XT_OFF + q * 512:XT_OFF + (q + 1) * 512]
                        .bitcast(fp8),
                        start=True, stop=True)
                    nc.scalar.activation(
                        hq[:, q * 512:(q + 1) * 512], ht[:],
                        AF.Tanh, bias=b1_t[:, 0:1])
            if do_lg:
                # ACT: exp right after the logit sweep completes
                ee = ee_pool.tile([128, TPC], bf16, name=f"ee{cl}", tag="ee")
                nc.scalar.activation(ee[:], lg, AF.Exp, bias=b2_t[:, 0:1])
                # DVE: wide me build
                sgl = st_dma[cl]
                me01 = me01_pool.tile([128, TPC * MSLOT], bf16,
                                      name=f"me01_{cl}", tag="me01")
                sg_b = (sgl[:, SG_OFF:SG_OFF + 2 * TPC].bitcast(bf16)
                        .unsqueeze(2).broadcast_to((128, TPC, MSLOT)))
                iota_b = iota_t[:, :].unsqueeze(1).broadcast_to((128, TPC, MSLOT))
                me01_3d = me01[:].rearrange("p (t s) -> p t s", t=TPC)
                nc.vector.tensor_tensor(me01_3d, sg_b, iota_b, op=ALU.is_equal)
                men = me_pool.tile([128, TPC * MSLOT], bf16,
                                   name=f"me{cl}", tag="me")
                ee_b = ee[:, :].unsqueeze(2).broadcast_to((128, TPC, MSLOT))
                me_3d = men[:].rearrange("p (t s) -> p t s", t=TPC)
                nc.vector.tensor_tensor(me_3d, me01_3d, ee_b, op=ALU.mult)
                st_me[cl] = men
                st_mlp.pop(cl, None)
            if do_pl:
                cp = i - LAG
                xap, me = st_dma[cp], st_me[cp]
                strip = 64 * (cp % 2)
                pl = st_pl[cp]
                for t in range(TPC):
                    nc.tensor.matmul(
                        pl[strip:strip + MSLOT, 0:H + 1],
                        lhsT=me[:, t * MSLOT:(t + 1) * MSLOT],
                        rhs=xap[:, XN_OFF + 2 * t * (H + 1):
                                XN_OFF + 2 * (t + 1) * (H + 1)].bitcast(bf16),
                        start=(t == 0), stop=(t == TPC - 1),
                        tile_position=(0, strip),
                        skip_group_check=True)
                nc.vector.tensor_copy(
                    wres_cols[cp // 2][strip:strip + MSLOT, :],
                    pl[strip:strip + MSLOT, 0:H + 1])
                for Gi in ready_groups.get(cp, ()):
                    emit_stitch(Gi)
                st_me.pop(cp, None)
                st_dma.pop(cp, None)
                st_pl.pop(cp, None)

    nc.compile()
    return nc


def kernel(x, batch, W1, b1, W2, b2):
    global LAST_EXEC_NS
    import os
    from concourse.bass_utils import run_bass_kernel_spmd

    x = np.asarray(x)
    batch = np.asarray(batch)
    shared, cores = _preprocess(x, batch, W1, b1, W2, b2, N_GRAPHS)

    key = (shared["NWIN"], shared["NGRP"], tuple(shared["wlo_shared"]))
    nc = _PROGRAM_CACHE.get(key)
    if nc is None:
        nc = _build_program(shared["NWIN"], shared["NGRP"], shared["wlo_shared"])
        _PROGRAM_CACHE[key] = nc

    in_maps = []
    for d in cores:
        in_maps.append({
            "xall": d["xall_img"],
            "iota": shared["iota"], "w1b": shared["w1b"], "w2b": shared["w2b"],
            "b1c": shared["b1c"], "b2c": shared["b2c"], "ind": d["ind_img"],
        })
    trace = os.environ.get("ATTNPOOL_TRACE", "0") == "1"
    tmpdir = os.environ.get("ATTNPOOL_TMPDIR") or None
    res = run_bass_kernel_spmd(nc, in_maps, core_ids=list(range(N_CORES)),
                               trace=trace, tmpdir=tmpdir)
    if res.exec_time_ns is not None:
        LAST_EXEC_NS = res.exec_time_ns

    out = np.zeros((N_GRAPHS, H), dtype=np.float32)
    gsplit = shared["gsplit"]
    for c, d in enumerate(cores):
        out[gsplit[c]:gsplit[c + 1]] = res.results[c]["out"][:d["gc"]]
    out[shared["counts"] == 0] = 0.0
    return out


# revision 37
# speedup vs baseline: 1.2784x; 1.0046x over previous
"""AttentionPooling (segment softmax-pool) TRN2 kernel, 8-core SPMD. V2.

Self-contained: kernel(**inputs) -> np.ndarray [16384, 128] f32.

Math (shift-invariance of softmax; logits are O(1) so exp can't overflow):
  e_i   = exp(tanh(x_i @ W1 + b1) @ W2 + b2)
  out_g = (sum_{i in g} e_i x_i) / (sum_{i in g} e_i)

Sharding: graphs are split into 8 contiguous ranges with ~equal node counts
(each graph's nodes land on one core); each core computes its own rows of the
output; host concatenates.

V2 device algorithm per core: x streams once in each of two bf16 layouts
(transposed [128, M] for the MLP, natural [x|1]-tiles for pooling). Per 4096-row
chunk: 8 matmuls W1^T@xT -> PSUM, tanh on [128,1024] spans, 32 per-tile
(lhsT=hq, rhs=w2, N=1) matmuls -> per-node logits in [128,32], one Exp, then the
masked-e matrix me[p, t*S+s] = e*(seg==s) is built with two WIDE tensor_tensor
ops per chunk using stride-0 broadcast APs (vs per-tile tensor_scalar in V1).
Pool matmuls (lhsT=me-tile, rhs=[x|1]) accumulate [48,129] per window; a static
indicator matmul re-bins window-slots to segments; reciprocal+scale normalizes.
Logit and pool matmuls are interleaved to keep the PE streaming (HAM warm).
"""

import math

import numpy as np
import ml_dtypes

BF16 = ml_dtypes.bfloat16
FP8 = ml_dtypes.float8_e3m4   # xT stream dtype (W1 stays bf16: mixed matmul)

N_CORES = 8
N_GRAPHS = 16384
H = 128
TILE = 128
TPC = 32             # tiles per window
CHUNK = TILE * TPC   # 4096 rows
SLOTS = 64           # slot stride per window in stitch space
NW_STITCH = 8        # stitch window span (static)
NCHK = NW_STITCH * SLOTS // 128
MSLOT = 44           # active slot width (real data max is 36/window)
PAD_SEG = 128.0      # any value >= MSLOT, exactly representable in bf16

LAST_EXEC_NS = None
_PROGRAM_CACHE = {}


# ---------------------------------------------------------------- host prep
def _preprocess(x, batch, W1, b1, W2, b2, n_graphs):
    N = x.shape[0]
    counts = np.bincount(batch, minlength=n_graphs)
    cum = np.zeros(n_graphs + 1, dtype=np.int64)
    np.cumsum(counts, out=cum[1:])

    gsplit = [0]
    for c in range(1, N_CORES):
        t = round(c * N / N_CORES)
        g = int(np.searchsorted(cum, t))
        if g > 0 and abs(cum[g - 1] - t) <= abs(cum[g] - t):
            g -= 1
        g = max(g, gsplit[-1] + 1)
        gsplit.append(min(g, n_graphs - (N_CORES - c)))
    gsplit.append(n_graphs)
    gsplit = np.array(gsplit, dtype=np.int64)

    Mc = [int(cum[gsplit[c + 1]] - cum[gsplit[c]]) for c in range(N_CORES)]
    Gc = [int(gsplit[c + 1] - gsplit[c]) for c in range(N_CORES)]
    NWIN = max(NW_STITCH, math.ceil(max(Mc) / CHUNK))
    if NWIN % 2:
        NWIN += 1
    M_pad = NWIN * CHUNK
    NGRP = math.ceil(max(Gc) / 128)

    x = np.asarray(x, dtype=np.float32)
    batch = np.asarray(batch)

    cores = []
    minw = np.full((N_CORES, NGRP), 10 ** 9, dtype=np.int64)
    maxw = np.full((N_CORES, NGRP), -1, dtype=np.int64)
    for c in range(N_CORES):
        nlo = int(cum[gsplit[c]])
        nhi = int(cum[gsplit[c + 1]])
        m = Mc[c]
        bl = batch[nlo:nhi].astype(np.int64) - gsplit[c]
        wfs = np.zeros(NWIN, dtype=np.int64)
        for w in range(NWIN):
            wfs[w] = bl[w * CHUNK] if w * CHUNK < m else Gc[c]
        slots = bl - wfs[np.arange(m) // CHUNK]
        assert slots.min() >= 0 and slots.max() < MSLOT, (
            f"core {c}: window slot range {slots.min()}..{slots.max()}")

        seg = np.full(M_pad, PAD_SEG, dtype=np.float32)
        seg[:m] = slots.astype(np.float32)
        seg_img = np.ascontiguousarray(seg.reshape(-1, TILE).T.astype(BF16))

        nT = M_pad // TILE
        xn = np.zeros((M_pad, H + 1), dtype=BF16)
        xn[:m, :H] = x[nlo:nhi]
        xn[:, H] = 1.0
        xn_img = np.ascontiguousarray(
            xn.reshape(nT, TILE, H + 1).transpose(1, 0, 2)
            .reshape(TILE, nT * (H + 1)))

        xt = np.zeros((M_pad, H), dtype=FP8)
        xt[:m] = x[nlo:nhi].astype(FP8)
        xt_img = np.ascontiguousarray(xt.T)

        # one interleaved byte stream per chunk: [xT fp8 | xn bf16 | seg bf16]
        xall_img = np.ascontiguousarray(np.concatenate([
            xt_img.view(np.uint8).reshape(TILE, NWIN, CHUNK),
            xn_img.view(np.uint8).reshape(TILE, NWIN, 2 * TPC * (H + 1)),
            seg_img.view(np.uint8).reshape(TILE, NWIN, 2 * TPC),
        ], axis=2).reshape(TILE, -1))

        lo_g = cum[gsplit[c]:gsplit[c + 1]] - nlo
        hi_g = cum[gsplit[c] + 1:gsplit[c + 1] + 1] - nlo
        nonempty = hi_g > lo_g
        wlo_g = np.where(nonempty, lo_g // CHUNK, 0)
        whi_g = np.where(nonempty, np.maximum(hi_g - 1, 0) // CHUNK, 0)
        for Gi in range(NGRP):
            a, b = Gi * 128, min(Gi * 128 + 128, Gc[c])
            if a >= Gc[c]:
                continue
            ne = nonempty[a:b]
            if ne.any():
                minw[c, Gi] = wlo_g[a:b][ne].min()
                maxw[c, Gi] = whi_g[a:b][ne].max()
        cores.append(dict(m=m, gc=Gc[c], wfs=wfs, xall_img=xall_img,
                          nonempty=nonempty, wlo_g=wlo_g, whi_g=whi_g))

    wlo_shared = []
    for Gi in range(NGRP):
        mn = int(minw[:, Gi].min())
        if mn >= 10 ** 9:
            mn = 0
        mn -= mn % 2
        mn = max(0, min(mn, NWIN - NW_STITCH))
        wlo_shared.append(mn)
        mx = int(maxw[:, Gi].max())
        assert mx < 0 or mx - mn + 1 <= NW_STITCH, (
            f"group {Gi}: window span {mn}..{mx} exceeds {NW_STITCH}")

    for c in range(N_CORES):
        d = cores[c]
        ind = np.zeros((128, NGRP * NCHK * 128), dtype=np.float32)
        for g in range(d["gc"]):
            if not d["nonempty"][g]:
                continue
            Gi = g // 128
            base_ws = wlo_shared[Gi] * SLOTS
            for w in range(int(d["wlo_g"][g]), int(d["whi_g"][g]) + 1):
                s = int(g - d["wfs"][w])
                wsl = w * SLOTS + s - base_ws
                assert 0 <= wsl < NW_STITCH * SLOTS
                ind[wsl % 128, (Gi * NCHK + wsl // 128) * 128 + (g - Gi * 128)] = 1.0
        d["ind_img"] = ind

    shared = dict(
        NWIN=NWIN, M_pad=M_pad, NGRP=NGRP, wlo_shared=wlo_shared,
        gsplit=gsplit, counts=counts,
        iota=np.ascontiguousarray(
            np.broadcast_to(np.arange(MSLOT, dtype=BF16), (128, MSLOT))),
        w1b=np.ascontiguousarray(np.asarray(W1).astype(BF16)),
        w2b=np.ascontiguousarray(np.asarray(W2).astype(BF16)),
        b1c=np.ascontiguousarray(np.asarray(b1).reshape(H, 1).astype(np.float32)),
        b2c=np.full((128, 1), np.asarray(b2).reshape(-1)[0], dtype=np.float32),
    )
    return shared, cores


# ---------------------------------------------------------------- program
def _build_program(NWIN, NGRP, wlo_shared,
                   xall_bufs=7, hq_bufs=4, me_bufs=5, me01_bufs=2):
    from contextlib import ExitStack
    import concourse.bacc as bacc
    import concourse.tile as tile
    from concourse import mybir

    M_pad = NWIN * CHUNK
    nT = M_pad // TILE
    NWCOL = NWIN * SLOTS // 128
    # interleaved per-chunk byte stream: [xT fp8 | xn bf16 | seg bf16]
    XAW = CHUNK + 2 * TPC * (H + 1) + 2 * TPC
    XT_OFF, XN_OFF, SG_OFF = 0, CHUNK, CHUNK + 2 * TPC * (H + 1)

    f32 = mybir.dt.float32
    bf16 = mybir.dt.bfloat16
    fp8 = mybir.dt.float8e3
    u8 = mybir.dt.uint8
    AF = mybir.ActivationFunctionType
    ALU = mybir.AluOpType

    nc = bacc.Bacc("TRN2", target_bir_lowering=False, debug=False,
                   enable_asserts=False, num_devices=N_CORES)
    xall_ap = nc.dram_tensor("xall", [128, NWIN * XAW], u8,
                             kind="ExternalInput").ap()
    iota_ap = nc.dram_tensor("iota", [128, MSLOT], bf16, kind="ExternalInput").ap()
    w1_ap = nc.dram_tensor("w1b", [128, H], bf16, kind="ExternalInput").ap()
    w2_ap = nc.dram_tensor("w2b", [128, 1], bf16, kind="ExternalInput").ap()
    b1_ap = nc.dram_tensor("b1c", [128, 1], f32, kind="ExternalInput").ap()
    b2_ap = nc.dram_tensor("b2c", [128, 1], f32, kind="ExternalInput").ap()
    ind_ap = nc.dram_tensor("ind", [128, NGRP * NCHK * 128], f32,
                            kind="ExternalInput").ap()
    out_ap = nc.dram_tensor("out", [NGRP * 128, H], f32, kind="ExternalOutput").ap()

    with tile.TileContext(nc) as tc, ExitStack() as ctx:
        consts = ctx.enter_context(tc.tile_pool(name="consts", bufs=1))
        xall_pool = ctx.enter_context(tc.tile_pool(name="xall", bufs=xall_bufs))
        hq_pool = ctx.enter_context(tc.tile_pool(name="hqp", bufs=hq_bufs))
        ee_pool = ctx.enter_context(tc.tile_pool(name="eep", bufs=4))
        me01_pool = ctx.enter_context(tc.tile_pool(name="me01p", bufs=me01_bufs))
        me_pool = ctx.enter_context(tc.tile_pool(name="mep", bufs=me_bufs))
        wres_pool = ctx.enter_context(tc.tile_pool(name="wres", bufs=1))
        ind_pool = ctx.enter_context(tc.tile_pool(name="indp", bufs=4))
        r_pool = ctx.enter_context(tc.tile_pool(name="rp", bufs=2))
        ob_pool = ctx.enter_context(tc.tile_pool(name="obp", bufs=2))
        ht_psum = ctx.enter_context(tc.tile_pool(name="htps", bufs=4, space="PSUM"))
        pl_psum = ctx.enter_context(tc.tile_pool(name="plps", bufs=4, space="PSUM"))

        iota_t = consts.tile([128, MSLOT], bf16, tag="iota")
        nc.sync.dma_start(iota_t[:], iota_ap[:])
        w1_t = consts.tile([128, H], bf16, tag="w1")
        nc.sync.dma_start(w1_t[:], w1_ap[:])
        w2_t = consts.tile([128, 1], bf16, tag="w2")
        nc.sync.dma_start(w2_t[:], w2_ap[:])
        b1_t = consts.tile([128, 1], f32, tag="b1")
        nc.sync.dma_start(b1_t[:], b1_ap[:])
        b2_t = consts.tile([128, 1], f32, tag="b2")
        nc.sync.dma_start(b2_t[:], b2_ap[:])
        wres_cols = [wres_pool.tile([128, H + 1], f32, name=f"wres{i}",
                                    tag=f"wres{i}")
                     for i in range(NWCOL)]
        for i in range(NWCOL):
            nc.vector.memset(wres_cols[i][:], 0.0)

        # emit group Gi's stitch right after its last window is flushed
        ready_groups = {}
        for Gi in range(NGRP):
            ready_groups.setdefault(wlo_shared[Gi] + NW_STITCH - 1, []).append(Gi)

        def emit_stitch(Gi):
            st = pl_psum.tile([128, H + 33], f32, tag="pl")
            for k in range(NCHK):
                it = ind_pool.tile([128, 128], f32)
                nc.sync.dma_start(
                    it[:], ind_ap[:, (Gi * NCHK + k) * 128:(Gi * NCHK + k + 1) * 128])
                wc = wlo_shared[Gi] // 2 + k
                nc.tensor.matmul(st[:, 0:H + 1], lhsT=it[:], rhs=wres_cols[wc][:],
                                 start=(k == 0), stop=(k == NCHK - 1))
            r = r_pool.tile([128, 1], f32)
            nc.vector.reciprocal(r[:], st[:, H:H + 1])
            ob = ob_pool.tile([128, H], f32)
            nc.vector.tensor_scalar(ob[:], st[:, 0:H], r[:, 0:1], None,
                                    op0=ALU.mult)
            nc.sync.dma_start(out_ap[Gi * 128:(Gi + 1) * 128, :], ob[:])

        # ---- software pipeline over chunks -------------------------------
        # Iteration i emits, in order:
        #   PE : logit(i-1) x32 interleaved with MLP(i) x8   (dense stream)
        #   ACT: exp(i-1) first, then tanh(i) x4  (so the me chain isn't
        #        stuck behind 4us of tanh)
        #   DVE: me01(i-1), me(i-1)  (wide TTs via stride-0 broadcast APs)
        #   PE : pool(i-3) x32  (3-deep lag so me is always ready)
        st_dma = {}
        st_mlp = {}
        st_me = {}
        st_pl = {}
        LAG = 4
        for i in range(NWIN + LAG):
            do_mlp = i < NWIN
            do_lg = 1 <= i <= NWIN
            do_pl = LAG <= i <= NWIN + LAG - 1
            if do_mlp:
                xall = xall_pool.tile([128, XAW], u8, name=f"xa{i}", tag="xa")
                nc.sync.dma_start(xall[:], xall_ap[:, i * XAW:(i + 1) * XAW])
                st_dma[i] = xall
                hq = hq_pool.tile([128, CHUNK], bf16, name=f"hq{i}", tag="hq")
                st_mlp[i] = hq
            if do_lg:
                cl = i - 1
                hql = st_mlp[cl]
                # logits live in cols H+1 .. H+32 of chunk cl's pl tile
                plc = pl_psum.tile([128, H + 33], f32, name=f"pl{cl}", tag="pl")
                st_pl[cl] = plc
                lg = plc[:, H + 1:H + 33]
            # PE: interleave logit(i-1) (weight-load heavy, N=1) with MLP(i)
            # (N=512 streams) so the array streams while hq weights load.
            for t in range(TPC):
                if do_lg:
                    nc.tensor.matmul(lg[:, t:t + 1],
                                     lhsT=hql[:, t * 128:(t + 1) * 128],
                                     rhs=w2_t[:], start=True, stop=True,
                                     skip_group_check=True)
                if do_mlp and t % 4 == 0:
                    q = t // 4
                    ht = ht_psum.tile([128, 512], f32,
                                      name=f"ht{i}_{q}", tag="ht")
                    nc.tensor.matmul(
                        ht[:], lhsT=w1_t[:],
                        rhs=xall[:, XT_OFF + q * 512:XT_OFF + (q + 1) * 512]
                        .bitcast(fp8),
                        start=True, stop=True)
                    nc.scalar.activation(
                        hq[:, q * 512:(q + 1) * 512], ht[:],
                        AF.Tanh, bias=b1_t[:, 0:1])
            if do_lg:
                # ACT: exp right after the logit sweep completes
                ee = ee_pool.tile([128, TPC], bf16, name=f"ee{cl}", tag="ee")
                nc.scalar.activation(ee[:], lg, AF.Exp, bias=b2_t[:, 0:1])
                # DVE: wide me build
                sgl = st_dma[cl]
                me01 = me01_pool.tile([128, TPC * MSLOT], bf16,
                                      name=f"me01_{cl}", tag="me01")
                sg_b = (sgl[:, SG_OFF:SG_OFF + 2 * TPC].bitcast(bf16)
                        .unsqueeze(2).broadcast_to((128, TPC, MSLOT)))
                iota_b = iota_t[:, :].unsqueeze(1).broadcast_to((128, TPC, MSLOT))
                me01_3d = me01[:].rearrange("p (t s) -> p t s", t=TPC)
                nc.vector.tensor_tensor(me01_3d, sg_b, iota_b, op=ALU.is_equal)
                men = me_pool.tile([128, TPC * MSLOT], bf16,
                                   name=f"me{cl}", tag="me")
                ee_b = ee[:, :].unsqueeze(2).broadcast_to((128, TPC, MSLOT))
                me_3d = men[:].rearrange("p (t s) -> p t s", t=TPC)
                nc.vector.tensor_tensor(me_3d, me01_3d, ee_b, op=ALU.mult)
                st_me[cl] = men
                st_mlp.pop(cl, None)
            if do_pl:
                cp = i - LAG
                xap, me = st_dma[cp], st_me[cp]
                strip = 64 * (cp % 2)
                pl = st_pl[cp]
                for t in range(TPC):
                    nc.tensor.matmul(
                        pl[strip:strip + MSLOT, 0:H + 1],
                        lhsT=me[:, t * MSLOT:(t + 1) * MSLOT],
                        rhs=xap[:, XN_OFF + 2 * t * (H + 1):
                                XN_OFF + 2 * (t + 1) * (H + 1)].bitcast(bf16),
                        start=(t == 0), stop=(t == TPC - 1),
                        tile_position=(0, strip),
                        skip_group_check=True)
                nc.vector.tensor_copy(
                    wres_cols[cp // 2][strip:strip + MSLOT, :],
                    pl[strip:strip + MSLOT, 0:H + 1])
                for Gi in ready_groups.get(cp, ()):
                    emit_stitch(Gi)
                st_me.pop(cp, None)
                st_dma.pop(cp, None)
                st_pl.pop(cp, None)

    nc.compile()
    return nc


def kernel(x, batch, W1, b1, W2, b2):
    global LAST_EXEC_NS
    import os
    from concourse.bass_utils import run_bass_kernel_spmd

    x = np.asarray(x)
    batch = np.asarray(batch)
    shared, cores = _preprocess(x, batch, W1, b1, W2, b2, N_GRAPHS)

    key = (shared["NWIN"], shared["NGRP"], tuple(shared["wlo_shared"]))
    nc = _PROGRAM_CACHE.get(key)
    if nc is None:
        nc = _build_program(shared["NWIN"], shared["NGRP"], shared["wlo_shared"])
        _PROGRAM_CACHE[key] = nc

    in_maps = []
    for d in cores:
        in_maps.append({
            "xall": d["xall_img"],
            "iota": shared["iota"], "w1b": shared["w1b"], "w2b": shared["w2b"],
            "b1c": shared["b1c"], "b2c": shared["b2c"], "ind": d["ind_img"],
        })
    trace = os.environ.get("ATTNPOOL_TRACE", "0") == "1"
    tmpdir = os.environ.get("ATTNPOOL_TMPDIR") or None
    res = run_bass_kernel_spmd(nc, in_maps, core_ids=list(range(N_CORES)),
                               trace=trace, tmpdir=tmpdir)
    if res.exec_time_ns is not None:
        LAST_EXEC_NS = res.exec_time_ns

    out = np.zeros((N_GRAPHS, H), dtype=np.float32)
    gsplit = shared["gsplit"]
    for c, d in enumerate(cores):
        out[gsplit[c]:gsplit[c + 1]] = res.results[c]["out"][:d["gc"]]
    out[shared["counts"] == 0] = 0.0
    return out
